# revision 1
# baseline (speedup 1.0000x reference)
"""AttentionHead with positional-bias matrices, 8-core Trainium2 Bass kernel.

Math (per reference):
  q = query @ Wq.T + bq           [B,S,D]
  k = key   @ Wk.T + bk           [B,S,D]
  v = value @ Wv.T + bv           [B,S,D]
  scores[b,s,t] = (q[b,s]·k[b,t] + q[b,s]·k_bias[s,t]) / sqrt(D) + maskadd[b,t]
  w = softmax_t(scores)
  out[b,s,:] = w[b,s,:] @ v[b] + sum_t w[b,s,t]*v_bias[s,t,:]

Sharding: sequence-parallel over the query-position axis s. Core c owns
s in [c*128, (c+1)*128) for ALL batches. The [S,S,D] bias matrices are
read exactly once globally (each core reads only its s-slice). k/v
projections are computed redundantly on every core (no collectives).

Host-side prep: downcast to bf16 (halves HBM traffic; matmuls accumulate
in f32 PSUM) and pre-transpose activations to [H, B*S] so the contraction
dim h lands on SBUF partitions without any on-device transposes.
1/sqrt(D) is folded into Wq/bq on the host.
"""

import os
import math
import numpy as np
import ml_dtypes

import concourse.bass as bass
import concourse.mybir as mybir
import concourse.tile as tile
from concourse import bacc
from concourse.masks import make_identity
from concourse.bass_utils import run_bass_kernel_spmd

B, S, H, D = 16, 1024, 1024, 128
NCORES = 8
SSL = S // NCORES          # query positions per core
BS = B * S                 # 16384
BSL = B * SSL              # 2048
HO = H // 128              # 8 h-chunks
TC = S // 128              # 8 t-chunks
PCHUNK = 512               # projection (b,t) chunk
NPCH = BS // PCHUNK        # 32
NQCH = BSL // PCHUNK       # 4

BF16 = mybir.dt.bfloat16
F32 = mybir.dt.float32

_cache = {}


def _build_proj_nc():
    """Launch 1: data-parallel q/k/v projection; core handles 2 batches.
    Outputs qT/kT in [d, (b_local, t)] layout and v in [tp, b_local, tc, d]."""
    nc = bacc.Bacc()
    NB = 2
    NCH = NB * S // PCHUNK  # 4 chunks per tensor

    xTs = {k: nc.dram_tensor(f"{k}T", [H, NB * S], BF16, kind="ExternalInput")
           for k in ("q", "k", "v")}
    Ws = {k: nc.dram_tensor(f"W{k}T", [H, D], BF16, kind="ExternalInput")
          for k in ("q", "k", "v")}
    bs = {k: nc.dram_tensor(f"b{k}", [D], F32, kind="ExternalInput")
          for k in ("q", "k", "v")}
    qTo = nc.dram_tensor("qTo", [128, NB * S], BF16, kind="ExternalOutput")
    kTo = nc.dram_tensor("kTo", [128, NB * S], BF16, kind="ExternalOutput")
    vo = nc.dram_tensor("vo", [128, NB, TC, D], BF16, kind="ExternalOutput")

    with tile.TileContext(nc) as tc:
        with (
            tc.tile_pool(name="const", bufs=1) as constp,
            tc.tile_pool(name="stream", bufs=3) as streamp,
            tc.tile_pool(name="evac", bufs=3) as evacp,
            tc.tile_pool(name="mmps", bufs=3, space="PSUM") as mmps,
            tc.tile_pool(name="tps", bufs=2, space="PSUM") as tps,
        ):
            ident = constp.tile([128, 128], BF16)
            make_identity(nc, ident[:])
            w_sb, b_sb = {}, {}
            for k in ("q", "k", "v"):
                w_sb[k] = constp.tile([128, HO, D], BF16, name=f"w_{k}", tag=f"w_{k}")
                nc.sync.dma_start(w_sb[k][:], Ws[k].rearrange("(ho p) d -> p ho d", p=128))
                b_sb[k] = constp.tile([128, 1], F32, name=f"b_{k}", tag=f"b_{k}")
                nc.sync.dma_start(b_sb[k][:], bs[k].rearrange("(o p) -> p o", p=128))
            vo_sb = constp.tile([128, NB, TC, D], BF16)

            for k in ("q", "k", "v"):
                src = xTs[k].rearrange("(ho p) n -> p ho n", p=128)
                for c in range(NCH):
                    xt = streamp.tile([128, HO, PCHUNK], BF16, tag="xchunk")
                    nc.sync.dma_start(xt[:], src[:, :, c * PCHUNK:(c + 1) * PCHUNK])
                    ps = mmps.tile([128, PCHUNK], F32, tag="mm")
                    for ho in range(HO):
                        nc.tensor.matmul(ps[:], lhsT=w_sb[k][:, ho, :],
                                         rhs=xt[:, ho, :],
                                         start=(ho == 0), stop=(ho == HO - 1))
                    if k in ("q", "k"):
                        ev = evacp.tile([128, PCHUNK], BF16, tag="ev")
                        nc.scalar.activation(ev[:], ps[:],
                                             mybir.ActivationFunctionType.Identity,
                                             bias=b_sb[k][:], scale=1.0)
                        dst = qTo if k == "q" else kTo
                        nc.sync.dma_start(dst[:, c * PCHUNK:(c + 1) * PCHUNK], ev[:])
                    else:
                        vt = evacp.tile([128, PCHUNK], BF16, tag="vt")
                        nc.scalar.activation(vt[:], ps[:],
                                             mybir.ActivationFunctionType.Identity,
                                             bias=b_sb[k][:], scale=1.0)
                        bl = c // 2
                        for i in range(PCHUNK // 128):
                            tcg = (c % 2) * 4 + i
                            tp_ps = tps.tile([128, 128], BF16, tag="tp")
                            nc.tensor.transpose(tp_ps[:], vt[:, i * 128:(i + 1) * 128],
                                                ident[:])
                            nc.vector.tensor_copy(out=vo_sb[:, bl, tcg, :], in_=tp_ps[:])
            nc.sync.dma_start(vo[:], vo_sb[:])
    nc.finalize()
    return nc


def _build_nc(mask_allones=True):
    nc = bacc.Bacc()

    # ---- per-core inputs (bf16 unless noted), all pre-projected/permuted ----
    qT_in = nc.dram_tensor("qT_in", [128, B, SSL], BF16, kind="ExternalInput")
    kT_in = nc.dram_tensor("kT_in", [128, B * S], BF16, kind="ExternalInput")
    v_in = nc.dram_tensor("v_in", [128, B, TC, D], BF16, kind="ExternalInput")
    kbT = nc.dram_tensor("kbT", [SSL, D, S], BF16, kind="ExternalInput")
    # vb host-permuted: [g, tp, sl, tc, d] with s = 2g+sl, t = tc*128+tp
    vb = nc.dram_tensor("vb", [SSL // 2, 128, 2, TC, D], BF16, kind="ExternalInput")
    maskadd = nc.dram_tensor("maskadd", [B, S], F32, kind="ExternalInput")
    out_h = nc.dram_tensor("out", [B, SSL, D], F32, kind="ExternalOutput")

    with tile.TileContext(nc) as tc:
        with (
            tc.tile_pool(name="const", bufs=1) as constp,
            tc.tile_pool(name="big", bufs=1) as bigp,
            tc.tile_pool(name="stream", bufs=2) as streamp,
            tc.tile_pool(name="evac", bufs=3) as evacp,
        ):
            # ---- resident SBUF tensors ----
            kT_sb = bigp.tile([128, B, S], BF16)          # [d, b, t]    32KB/part
            v_sb = bigp.tile([128, B, TC, 128], BF16)     # [tp, b, tc, d] 32KB
            qT_sb = bigp.tile([128, B, SSL], BF16)        # [d, b, s]    4KB
            a2buf = bigp.tile([128, B, S], BF16)          # [s, b, t]    32KB
            eT_sb = bigp.tile([128, TC, B, SSL], BF16)    # [tp, tc, b, s] 32KB
            v2buf = bigp.tile([128, B, D], BF16)          # [s, b, d]    4KB
            outbuf = bigp.tile([128, B, D], F32)          # [s, b, d]    8KB
            rowsum = bigp.tile([128, B], F32)
            recip = bigp.tile([128, B], F32)

            mask_sb = constp.tile([B, S], F32)
            ident = constp.tile([128, 128], BF16)
            nc.sync.dma_start(mask_sb[:], maskadd[:, :])
            make_identity(nc, ident[:])

            # qT on SP (needed first, by attn_2); the big kT/v preloads go on
            # the ACT HWDGE queue so the kbT stream isn't queued behind them
            nc.sync.dma_start(qT_sb[:], qT_in[:, :, :])
            nc.scalar.dma_start(kT_sb.rearrange("p b t -> p (b t)")[:], kT_in[:, :])
            nc.scalar.dma_start(v_sb[:], v_in[:])

            # ========== P1: attn_2 per s: a2[b,t] = sum_d q[b,s,d]*kb[s,t,d]
            # kbT streamed in 2-s groups (1MB DMAs); shuffle DMAs on SWDGE
            with tc.tile_pool(name="a2ps", bufs=4, space="PSUM") as a2ps:
                for g in range(SSL // 2):
                    kbt = streamp.tile([128, 2, S], BF16, tag="kbt", bufs=3)
                    nc.sync.dma_start(kbt[:], kbT[2 * g:2 * g + 2].rearrange("s d t -> d s t"))
                    for si in range(2):
                        s = 2 * g + si
                        ps = a2ps.tile([B, S], F32, tag="a2")
                        for h in range(2):
                            nc.tensor.matmul(ps[:, h * 512:(h + 1) * 512],
                                             lhsT=qT_sb[:, :, s],
                                             rhs=kbt[:, si, h * 512:(h + 1) * 512],
                                             start=True, stop=True)
                        # evac (+ mask add broadcast over s) -> bf16
                        ev = evacp.tile([B, S], BF16, tag="a2evac")
                        if mask_allones:
                            if si % 2 == 0:
                                nc.vector.tensor_copy(out=ev[:], in_=ps[:])
                            else:
                                nc.scalar.copy(ev[:], ps[:])
                        else:
                            nc.vector.tensor_add(out=ev[:], in0=ps[:], in1=mask_sb[:])
                        # row-shuffle: [b, t] rows -> partition s of a2buf
                        # (split across SWDGE and ACT-HWDGE queues; keeps the
                        # SP queue free for the kbT stream)
                        if si % 2 == 0:
                            nc.gpsimd.dma_start(a2buf[s:s + 1, :, :], ev[:])
                        else:
                            nc.scalar.dma_start(a2buf[s:s + 1, :, :], ev[:])

            # ================= P3a: scores + softmax + eT, per b =================
            with (
                tc.tile_pool(name="scps", bufs=2, space="PSUM") as scps,
                tc.tile_pool(name="tps2", bufs=2, space="PSUM") as tps2,
            ):
                for b in range(B):
                    ps = scps.tile([128, S], F32, tag="sc")
                    for h in range(2):
                        sl = slice(h * 512, (h + 1) * 512)
                        nc.tensor.matmul(ps[:, sl], lhsT=qT_sb[:, b, :],
                                         rhs=kT_sb[:, b, sl], start=True, stop=False)
                        nc.tensor.matmul(ps[:, sl], lhsT=ident[:],
                                         rhs=a2buf[:, b, sl], start=False, stop=True)
                    e_sb = evacp.tile([128, S], BF16, tag="e", bufs=2)
                    nc.scalar.activation(e_sb[:], ps[:],
                                         mybir.ActivationFunctionType.Exp,
                                         bias=0.0, scale=1.0,
                                         accum_out=rowsum[:, b:b + 1])
                    for t in range(TC):
                        tp_ps = tps2.tile([128, 128], BF16, tag="tp2")
                        nc.tensor.transpose(tp_ps[:], e_sb[:, t * 128:(t + 1) * 128],
                                            ident[:])
                        nc.vector.tensor_copy(out=eT_sb[:, t, b, :], in_=tp_ps[:])
                nc.vector.reciprocal(recip[:], rowsum[:])

            # ================= P4: values_2 (bias values), per s =================
            # v2[b, d] = sum_t e[b, s, t] * v_bias[s, t, d]
            with tc.tile_pool(name="v2ps", bufs=4, space="PSUM") as v2ps:
                for g in range(SSL // 2):
                    vbt = streamp.tile([128, 2, TC, D], BF16, tag="vbt", bufs=4)
                    nc.sync.dma_start(vbt[:], vb[g])
                    for si in range(2):
                        s = 2 * g + si
                        ps = v2ps.tile([B, D], F32, tag="v2")
                        for t in range(TC):
                            nc.tensor.matmul(ps[:], lhsT=eT_sb[:, t, :, s],
                                             rhs=vbt[:, si, t, :],
                                             start=(t == 0), stop=(t == TC - 1))
                        ev = evacp.tile([B, D], BF16, tag="v2evac")
                        if si == 0:
                            nc.vector.tensor_copy(out=ev[:], in_=ps[:])
                            nc.gpsimd.dma_start(v2buf[s:s + 1, :, :], ev[:])
                        else:
                            nc.scalar.copy(ev[:], ps[:])
                            nc.scalar.dma_start(v2buf[s:s + 1, :, :], ev[:])

            # ================= P3b: values_1 + combine + out =================
            with tc.tile_pool(name="ops", bufs=2, space="PSUM") as ops:
                for b in range(B):
                    ps = ops.tile([128, D], F32, tag="o")
                    for t in range(TC):
                        nc.tensor.matmul(ps[:], lhsT=eT_sb[:, t, b, :],
                                         rhs=v_sb[:, b, t, :],
                                         start=(t == 0), stop=False)
                    nc.tensor.matmul(ps[:], lhsT=ident[:], rhs=v2buf[:, b, :],
                                     start=False, stop=True)
                    nc.scalar.activation(outbuf[:, b, :], ps[:],
                                         mybir.ActivationFunctionType.Copy,
                                         bias=0.0, scale=recip[:, b:b + 1])
                    # per-b store overlaps the remaining values_1 compute
                    nc.sync.dma_start(out_h[b].rearrange("s d -> s d"),
                                      outbuf[:, b, :])

    nc.finalize()
    return nc


def _prep_proj_inputs(query, key, value, Wq, bq, Wk, bk, Wv, bv):
    scale = 1.0 / math.sqrt(D)
    bf = ml_dtypes.bfloat16
    WqTs = np.ascontiguousarray((Wq.T * scale)).astype(bf)
    WkT = np.ascontiguousarray(Wk.T).astype(bf)
    WvT = np.ascontiguousarray(Wv.T).astype(bf)
    bqs = (bq * scale).astype(np.float32)
    in_maps = []
    for c in range(NCORES):
        bsl = slice(2 * c, 2 * c + 2)
        m = dict(WqT=WqTs, WkT=WkT, WvT=WvT,
                 bq=bqs, bk=bk.astype(np.float32), bv=bv.astype(np.float32))
        for nm, x in (("qT", query), ("kT", key), ("vT", value)):
            m[nm] = np.ascontiguousarray(
                x[bsl].transpose(2, 0, 1).reshape(H, 2 * S)).astype(bf)
        in_maps.append(m)
    return in_maps


def _prep_attn_inputs(proj_results, mask, k_bias, v_bias):
    bf = ml_dtypes.bfloat16
    # assemble full projected tensors from the 8 data-parallel shards
    qT_full = np.concatenate(  # [128, B, S]
        [r["qTo"].reshape(128, 2, S) for r in proj_results], axis=1)
    kT_full = np.concatenate(
        [r["kTo"].reshape(128, 2, S) for r in proj_results], axis=1)
    v_full = np.concatenate(  # [128, B, TC, D]
        [r["vo"] for r in proj_results], axis=1)
    kT_in = np.ascontiguousarray(kT_full.reshape(128, B * S))
    v_in = np.ascontiguousarray(v_full)
    maskadd = np.where(mask == 0, np.float32(-1e9), np.float32(0.0)).astype(np.float32)

    in_maps = []
    for c in range(NCORES):
        ssl = slice(c * SSL, (c + 1) * SSL)
        qT_in = np.ascontiguousarray(qT_full[:, :, ssl])
        kbT = np.ascontiguousarray(k_bias[ssl].transpose(0, 2, 1)).astype(bf)
        # vb: [s, t, d] -> [g, tp, sl, tc, d]  (s=2g+sl, t=tc*128+tp)
        vbc = np.ascontiguousarray(
            v_bias[ssl].reshape(SSL // 2, 2, TC, 128, D).transpose(0, 3, 1, 2, 4)
        ).astype(bf)
        in_maps.append(dict(qT_in=qT_in, kT_in=kT_in, v_in=v_in,
                            kbT=kbT, vb=vbc, maskadd=maskadd))
    return in_maps


def kernel(**inputs):
    ins = {k: np.asarray(v) for k, v in inputs.items()}
    allones = bool((ins["mask"] != 0).all())
    if "nc_proj" not in _cache:
        _cache["nc_proj"] = _build_proj_nc()
    key = f"nc{int(allones)}"
    if key not in _cache:
        _cache[key] = _build_nc(mask_allones=allones)
    nc = _cache[key]
    _cache["nc"] = nc

    proj_maps = _prep_proj_inputs(
        ins["query"], ins["key"], ins["value"], ins["Wq"], ins["bq"],
        ins["Wk"], ins["bk"], ins["Wv"], ins["bv"])
    _cache["proj_in_maps"] = proj_maps
    res1 = run_bass_kernel_spmd(_cache["nc_proj"], proj_maps,
                                core_ids=list(range(NCORES)))
    in_maps = _prep_attn_inputs(res1.results, ins["mask"], ins["k_bias"],
                                ins["v_bias"])
    _cache["attn_in_maps"] = in_maps
    res = run_bass_kernel_spmd(nc, in_maps, core_ids=list(range(NCORES)))
    out = np.concatenate([r["out"] for r in res.results], axis=1)
    return out



# revision 5
# speedup vs baseline: 1.0000x; 1.0000x over previous
"""AttentionHead with positional-bias matrices, 8-core Trainium2 Bass kernel.

Math (per reference):
  q = query @ Wq.T + bq           [B,S,D]
  k = key   @ Wk.T + bk           [B,S,D]
  v = value @ Wv.T + bv           [B,S,D]
  scores[b,s,t] = (q[b,s]·k[b,t] + q[b,s]·k_bias[s,t]) / sqrt(D) + maskadd[b,t]
  w = softmax_t(scores)
  out[b,s,:] = w[b,s,:] @ v[b] + sum_t w[b,s,t]*v_bias[s,t,:]

Sharding: sequence-parallel over the query-position axis s. Core c owns
s in [c*128, (c+1)*128) for ALL batches. The [S,S,D] bias matrices are
read exactly once globally (each core reads only its s-slice). k/v
projections are computed data-parallel in a first launch (2 batches per
core) and redistributed through the host.

Perf-critical choices vs the v1 kernel:
  - fp16 everywhere instead of bf16 (same bytes, ~10x less rounding noise),
    except e / eT which stay bf16 (exp output can exceed fp16 range).
  - k_bias: t in [0,512) stored fp16, t in [512,1024) stored fp8-e3m4;
    v_bias: t-tiles 0-5 e3m4, 6-7 fp16. Mixed-dtype matmuls (fp16 lhsT x
    fp8 rhs) run at full stream rate, so this halves most of the dominant
    HBM traffic at a measured ~1.4e-2 total rel-err.
  - P1 (attn_2) and P4 (values_2) use PE column tiling: 4 query positions
    run concurrently in separate 32-column groups of the PE array
    (tile_position=(0,32g)), lifting the M=16 matmuls from 12.5% to ~50%
    array utilization and making the PSUM evacuations full-width.
  - values_2 is written out raw (with rowsums) and normalized + added on
    the host, killing the v2 row-shuffle pass entirely.
"""

import math
import numpy as np
import ml_dtypes

import concourse.bass as bass
import concourse.mybir as mybir
import concourse.tile as tile
from concourse import bacc
from concourse.masks import make_identity
from concourse.bass_utils import run_bass_kernel_spmd

B, S, H, D = 16, 1024, 1024, 128
NCORES = 8
SSL = S // NCORES          # query positions per core (128)
BS = B * S                 # 16384
HO = H // 128              # 8 h-chunks
TC = S // 128              # 8 t-chunks
PCHUNK = 512               # projection (b,t) chunk
NG = SSL // 4              # 32 col-tiled 4-s groups
KB16 = 512                 # k_bias cols stored fp16 (rest e3m4)
VB8 = 6                    # v_bias t-tiles stored e3m4 (rest fp16)

F16 = mybir.dt.float16
E3 = mybir.dt.float8e3
BF16 = mybir.dt.bfloat16
F32 = mybir.dt.float32

_cache = {}


def _build_proj_nc():
    """Launch 1: data-parallel q/k/v projection; core handles 2 batches.
    Outputs qT/kT in [d, (b_local, t)] layout and v in [tp, b_local, tc, d]."""
    nc = bacc.Bacc()
    NB = 2
    NCH = NB * S // PCHUNK  # 4 chunks per tensor

    xTs = {k: nc.dram_tensor(f"{k}T", [H, NB * S], F16, kind="ExternalInput")
           for k in ("q", "k", "v")}
    Ws = {k: nc.dram_tensor(f"W{k}T", [H, D], F16, kind="ExternalInput")
          for k in ("q", "k", "v")}
    bs = {k: nc.dram_tensor(f"b{k}", [D], F32, kind="ExternalInput")
          for k in ("q", "k", "v")}
    qTo = nc.dram_tensor("qTo", [128, NB * S], F16, kind="ExternalOutput")
    kTo = nc.dram_tensor("kTo", [128, NB * S], F16, kind="ExternalOutput")
    vo = nc.dram_tensor("vo", [128, NB, TC, D], F16, kind="ExternalOutput")

    with tile.TileContext(nc) as tc:
        with (
            tc.tile_pool(name="const", bufs=1) as constp,
            tc.tile_pool(name="stream", bufs=3) as streamp,
            tc.tile_pool(name="evac", bufs=3) as evacp,
            tc.tile_pool(name="mmps", bufs=3, space="PSUM") as mmps,
            tc.tile_pool(name="tps", bufs=2, space="PSUM") as tps,
        ):
            ident = constp.tile([128, 128], F16)
            make_identity(nc, ident[:])
            w_sb, b_sb = {}, {}
            for k in ("q", "k", "v"):
                w_sb[k] = constp.tile([128, HO, D], F16, name=f"w_{k}", tag=f"w_{k}")
                nc.sync.dma_start(w_sb[k][:], Ws[k].rearrange("(ho p) d -> p ho d", p=128))
                b_sb[k] = constp.tile([128, 1], F32, name=f"b_{k}", tag=f"b_{k}")
                nc.sync.dma_start(b_sb[k][:], bs[k].rearrange("(o p) -> p o", p=128))
            vo_sb = constp.tile([128, NB, TC, D], F16)

            for k in ("q", "k", "v"):
                src = xTs[k].rearrange("(ho p) n -> p ho n", p=128)
                for c in range(NCH):
                    xt = streamp.tile([128, HO, PCHUNK], F16, tag="xchunk")
                    nc.sync.dma_start(xt[:], src[:, :, c * PCHUNK:(c + 1) * PCHUNK])
                    ps = mmps.tile([128, PCHUNK], F32, tag="mm")
                    for ho in range(HO):
                        nc.tensor.matmul(ps[:], lhsT=w_sb[k][:, ho, :],
                                         rhs=xt[:, ho, :],
                                         start=(ho == 0), stop=(ho == HO - 1))
                    if k in ("q", "k"):
                        ev = evacp.tile([128, PCHUNK], F16, tag="ev")
                        nc.scalar.activation(ev[:], ps[:],
                                             mybir.ActivationFunctionType.Identity,
                                             bias=b_sb[k][:], scale=1.0)
                        dst = qTo if k == "q" else kTo
                        nc.sync.dma_start(dst[:, c * PCHUNK:(c + 1) * PCHUNK], ev[:])
                    else:
                        vt = evacp.tile([128, PCHUNK], F16, tag="vt")
                        nc.scalar.activation(vt[:], ps[:],
                                             mybir.ActivationFunctionType.Identity,
                                             bias=b_sb[k][:], scale=1.0)
                        bl = c // 2
                        for i in range(PCHUNK // 128):
                            tcg = (c % 2) * 4 + i
                            tp_ps = tps.tile([128, 128], F16, tag="tp")
                            nc.tensor.transpose(tp_ps[:], vt[:, i * 128:(i + 1) * 128],
                                                ident[:])
                            nc.vector.tensor_copy(out=vo_sb[:, bl, tcg, :], in_=tp_ps[:])
            nc.sync.dma_start(vo[:], vo_sb[:])
    nc.finalize()
    return nc


def _build_nc(mask_allones=True):
    nc = bacc.Bacc()

    # ---- per-core inputs, all pre-projected/permuted host-side ----
    qT_in = nc.dram_tensor("qT_in", [128, B, SSL], F16, kind="ExternalInput")
    kT_in = nc.dram_tensor("kT_in", [128, B * S], F16, kind="ExternalInput")
    v_in = nc.dram_tensor("v_in", [128, B, TC, D], F16, kind="ExternalInput")
    # k_bias slice, pre-transposed to [d, s, t]; t split by precision
    kb16 = nc.dram_tensor("kb16", [128, SSL, KB16], F16, kind="ExternalInput")
    kb8 = nc.dram_tensor("kb8", [128, SSL, S - KB16], E3, kind="ExternalInput")
    # v_bias slice, pre-permuted to [tp, s, tc, d]; tc tiles split by precision
    vb8 = nc.dram_tensor("vb8", [128, SSL, VB8, D], E3, kind="ExternalInput")
    vb16 = nc.dram_tensor("vb16", [128, SSL, TC - VB8, D], F16, kind="ExternalInput")
    maskadd = nc.dram_tensor("maskadd", [B, S], F32, kind="ExternalInput")
    out_h = nc.dram_tensor("out", [B, SSL, D], F32, kind="ExternalOutput")
    v2o = nc.dram_tensor("v2o", [NG, 128, D], F32, kind="ExternalOutput")
    rso = nc.dram_tensor("rso", [SSL, B], F32, kind="ExternalOutput")

    with tile.TileContext(nc) as tc:
        with (
            tc.tile_pool(name="const", bufs=1) as constp,
            tc.tile_pool(name="big", bufs=1) as bigp,
            tc.tile_pool(name="stream", bufs=2) as streamp,
            tc.tile_pool(name="evac", bufs=3) as evacp,
        ):
            # ---- resident SBUF tensors ----
            kT_sb = bigp.tile([128, B, S], F16)           # [d, b, t]
            v_sb = bigp.tile([128, B, TC, 128], F16)      # [tp, b, tc, d]
            qT_sb = bigp.tile([128, B, SSL], F16)         # [d, b, s]
            a2buf = bigp.tile([128, B, S], F16)           # [s, b, t]
            eT_sb = bigp.tile([128, TC, B, SSL], BF16)    # [tp, tc, b, s]
            outbuf = bigp.tile([128, B, D], F32)          # [s, b, d]
            rowsum = bigp.tile([128, B], F32)
            recip = bigp.tile([128, B], F32)

            ident = constp.tile([128, 128], BF16)
            make_identity(nc, ident[:])
            ident16 = constp.tile([128, 128], F16)
            make_identity(nc, ident16[:])
            if not mask_allones:
                mask4 = constp.tile([128, S], F32)
                for g in range(4):
                    nc.scalar.dma_start(mask4[32 * g:32 * g + 16, :], maskadd[:, :])

            # qT first (needed by P1); big kT/v preloads on the ACT HWDGE
            # queue so the kb stream isn't queued behind them
            nc.sync.dma_start(qT_sb[:], qT_in[:, :, :])
            nc.scalar.dma_start(kT_sb.rearrange("p b t -> p (b t)")[:], kT_in[:, :])
            nc.scalar.dma_start(v_sb[:], v_in[:])

            # ========== P1: attn_2, col-tiled 4 s at a time ==========
            # a2[b,t] = sum_d q[b,s,d]*kb[s,t,d]; group G handles s=4G..4G+3,
            # each s in its own 32-col group of the PE array.
            shuffle_engines = [nc.gpsimd, nc.scalar, nc.sync]
            with tc.tile_pool(name="a2ps", bufs=3, space="PSUM") as a2ps:
                for G in range(NG):
                    s0 = 4 * G
                    kt16 = streamp.tile([128, 4, KB16], F16, tag="kbt16", bufs=2)
                    nc.sync.dma_start(kt16[:], kb16[:, s0:s0 + 4, :])
                    kt8 = streamp.tile([128, 4, S - KB16], E3, tag="kbt8", bufs=2)
                    nc.sync.dma_start(kt8[:], kb8[:, s0:s0 + 4, :])
                    ps = a2ps.tile([128, S], F32, tag="a2")
                    for g in range(4):
                        s = s0 + g
                        nc.tensor.matmul(ps[32 * g:32 * g + 16, 0:KB16],
                                         lhsT=qT_sb[:, :, s], rhs=kt16[:, g, :],
                                         start=True, stop=True,
                                         tile_position=(0, 32 * g))
                        nc.tensor.matmul(ps[32 * g:32 * g + 16, KB16:S],
                                         lhsT=qT_sb[:, :, s], rhs=kt8[:, g, :],
                                         start=True, stop=True,
                                         tile_position=(0, 32 * g))
                    ev = evacp.tile([128, S], F16, tag="a2evac")
                    if mask_allones:
                        if G % 2 == 0:
                            nc.vector.tensor_copy(out=ev[:], in_=ps[:])
                        else:
                            nc.scalar.copy(ev[:], ps[:])
                    else:
                        nc.vector.tensor_add(out=ev[:], in0=ps[:], in1=mask4[:])
                    # row-shuffle each s into partition s of a2buf
                    for g in range(4):
                        eng = shuffle_engines[(4 * G + g) % 3]
                        eng.dma_start(a2buf[s0 + g:s0 + g + 1, :, :],
                                      ev[32 * g:32 * g + 16, :])

            # ========== P3a: scores + softmax + eT, per b ==========
            with (
                tc.tile_pool(name="scps", bufs=2, space="PSUM") as scps,
                tc.tile_pool(name="tps2", bufs=2, space="PSUM") as tps2,
            ):
                for b in range(B):
                    ps = scps.tile([128, S], F32, tag="sc")
                    for h in range(2):
                        sl = slice(h * 512, (h + 1) * 512)
                        nc.tensor.matmul(ps[:, sl], lhsT=qT_sb[:, b, :],
                                         rhs=kT_sb[:, b, sl], start=True, stop=False)
                        nc.tensor.matmul(ps[:, sl], lhsT=ident16[:],
                                         rhs=a2buf[:, b, sl], start=False, stop=True)
                    e_sb = evacp.tile([128, S], BF16, tag="e", bufs=2)
                    nc.scalar.activation(e_sb[:], ps[:],
                                         mybir.ActivationFunctionType.Exp,
                                         bias=0.0, scale=1.0,
                                         accum_out=rowsum[:, b:b + 1])
                    for t in range(TC):
                        tp_ps = tps2.tile([128, 128], BF16, tag="tp2")
                        nc.tensor.transpose(tp_ps[:], e_sb[:, t * 128:(t + 1) * 128],
                                            ident[:])
                        nc.vector.tensor_copy(out=eT_sb[:, t, b, :], in_=tp_ps[:])
                nc.vector.reciprocal(recip[:], rowsum[:])
                nc.gpsimd.dma_start(rso[:, :], rowsum[:])

            # ========== P4: values_2 raw, col-tiled 4 s at a time ==========
            # v2[b,d] = sum_t e[b,s,t]*vb[s,t,d]; normalized + added on host.
            with tc.tile_pool(name="v2ps", bufs=4, space="PSUM") as v2ps:
                for G in range(NG):
                    s0 = 4 * G
                    vt8 = streamp.tile([128, 4, VB8, D], E3, tag="vbt8", bufs=2)
                    nc.sync.dma_start(vt8[:], vb8[:, s0:s0 + 4, :, :])
                    vt16 = streamp.tile([128, 4, TC - VB8, D], F16, tag="vbt16", bufs=2)
                    nc.sync.dma_start(vt16[:], vb16[:, s0:s0 + 4, :, :])
                    ps = v2ps.tile([128, D], F32, tag="v2")
                    for g in range(4):
                        s = s0 + g
                        for t in range(TC):
                            rhs = (vt8[:, g, t, :] if t < VB8
                                   else vt16[:, g, t - VB8, :])
                            nc.tensor.matmul(ps[32 * g:32 * g + 16, :],
                                             lhsT=eT_sb[:, t, :, s], rhs=rhs,
                                             start=(t == 0), stop=(t == TC - 1),
                                             tile_position=(0, 32 * g))
                    ev = evacp.tile([128, D], F32, tag="v2evac")
                    if G % 2 == 0:
                        nc.vector.tensor_copy(out=ev[:], in_=ps[:])
                    else:
                        nc.scalar.copy(ev[:], ps[:])
                    eng = shuffle_engines[G % 2]
                    eng.dma_start(v2o[G], ev[:])

            # ========== P3b: values_1 + normalize + out ==========
            with tc.tile_pool(name="ops", bufs=2, space="PSUM") as ops:
                for b in range(B):
                    ps = ops.tile([128, D], F32, tag="o")
                    for t in range(TC):
                        nc.tensor.matmul(ps[:], lhsT=eT_sb[:, t, b, :],
                                         rhs=v_sb[:, b, t, :],
                                         start=(t == 0), stop=(t == TC - 1))
                    nc.scalar.activation(outbuf[:, b, :], ps[:],
                                         mybir.ActivationFunctionType.Copy,
                                         bias=0.0, scale=recip[:, b:b + 1])
                    nc.sync.dma_start(out_h[b].rearrange("s d -> s d"),
                                      outbuf[:, b, :])

    nc.finalize()
    return nc


def _prep_proj_inputs(query, key, value, Wq, bq, Wk, bk, Wv, bv):
    scale = 1.0 / math.sqrt(D)
    f16 = np.float16
    WqTs = np.ascontiguousarray((Wq.T * scale)).astype(f16)
    WkT = np.ascontiguousarray(Wk.T).astype(f16)
    WvT = np.ascontiguousarray(Wv.T).astype(f16)
    bqs = (bq * scale).astype(np.float32)
    in_maps = []
    for c in range(NCORES):
        bsl = slice(2 * c, 2 * c + 2)
        m = dict(WqT=WqTs, WkT=WkT, WvT=WvT,
                 bq=bqs, bk=bk.astype(np.float32), bv=bv.astype(np.float32))
        for nm, x in (("qT", query), ("kT", key), ("vT", value)):
            m[nm] = np.ascontiguousarray(
                x[bsl].transpose(2, 0, 1).reshape(H, 2 * S)).astype(f16)
        in_maps.append(m)
    return in_maps


def _prep_attn_inputs(proj_results, mask, k_bias, v_bias):
    f16 = np.float16
    e3 = ml_dtypes.float8_e3m4
    qT_full = np.concatenate(  # [128, B, S]
        [r["qTo"].reshape(128, 2, S) for r in proj_results], axis=1)
    kT_full = np.concatenate(
        [r["kTo"].reshape(128, 2, S) for r in proj_results], axis=1)
    v_full = np.concatenate(  # [128, B, TC, D]
        [r["vo"] for r in proj_results], axis=1)
    kT_in = np.ascontiguousarray(kT_full.reshape(128, B * S))
    v_in = np.ascontiguousarray(v_full)
    maskadd = np.where(mask == 0, np.float32(-30000.0),
                       np.float32(0.0)).astype(np.float32)

    in_maps = []
    for c in range(NCORES):
        ssl = slice(c * SSL, (c + 1) * SSL)
        qT_in = np.ascontiguousarray(qT_full[:, :, ssl])
        kbT = k_bias[ssl].transpose(2, 0, 1)          # [d, s, t]
        kb16c = np.ascontiguousarray(kbT[:, :, :KB16]).astype(f16)
        kb8c = np.ascontiguousarray(kbT[:, :, KB16:]).astype(e3)
        vbp = v_bias[ssl].reshape(SSL, TC, 128, D).transpose(2, 0, 1, 3)
        vb8c = np.ascontiguousarray(vbp[:, :, :VB8, :]).astype(e3)
        vb16c = np.ascontiguousarray(vbp[:, :, VB8:, :]).astype(f16)
        in_maps.append(dict(qT_in=qT_in, kT_in=kT_in, v_in=v_in,
                            kb16=kb16c, kb8=kb8c, vb8=vb8c, vb16=vb16c,
                            maskadd=maskadd))
    return in_maps


def kernel(**inputs):
    ins = {k: np.asarray(v) for k, v in inputs.items()}
    allones = bool((ins["mask"] != 0).all())
    if "nc_proj" not in _cache:
        _cache["nc_proj"] = _build_proj_nc()
    key = f"nc{int(allones)}"
    if key not in _cache:
        _cache[key] = _build_nc(mask_allones=allones)
    nc = _cache[key]
    _cache["nc"] = nc

    proj_maps = _prep_proj_inputs(
        ins["query"], ins["key"], ins["value"], ins["Wq"], ins["bq"],
        ins["Wk"], ins["bk"], ins["Wv"], ins["bv"])
    _cache["proj_in_maps"] = proj_maps
    res1 = run_bass_kernel_spmd(_cache["nc_proj"], proj_maps,
                                core_ids=list(range(NCORES)))
    in_maps = _prep_attn_inputs(res1.results, ins["mask"], ins["k_bias"],
                                ins["v_bias"])
    _cache["attn_in_maps"] = in_maps
    res = run_bass_kernel_spmd(nc, in_maps, core_ids=list(range(NCORES)))

    # assemble: out = normalized values_1; add host-normalized values_2
    out = np.concatenate([r["out"] for r in res.results], axis=1)  # [B,S,D]
    for c in range(NCORES):
        v2 = res.results[c]["v2o"].reshape(NG, 4, 32, D)[:, :, :B, :]
        v2 = v2.transpose(2, 0, 1, 3).reshape(B, SSL, D)       # [b, s_local, d]
        rs = res.results[c]["rso"]                              # [s_local, b]
        out[:, c * SSL:(c + 1) * SSL, :] += v2 / rs.T[:, :, None]
    return out


# revision 12
# speedup vs baseline: 22611235568561.0195x; 22611235566377.8164x over previous
"""AttentionHead with positional-bias matrices, 8-core Trainium2 Bass kernel.

Math (per reference):
  q = query @ Wq.T + bq           [B,S,D]
  k = key   @ Wk.T + bk           [B,S,D]
  v = value @ Wv.T + bv           [B,S,D]
  scores[b,s,t] = (q[b,s]·k[b,t] + q[b,s]·k_bias[s,t]) / sqrt(D) + maskadd[b,t]
  w = softmax_t(scores)
  out[b,s,:] = w[b,s,:] @ v[b] + sum_t w[b,s,t]*v_bias[s,t,:]

Sharding: sequence-parallel over the query-position axis s. Core c owns
s in [c*128, (c+1)*128) for ALL batches. The [S,S,D] bias matrices are
read exactly once globally (each core reads only its s-slice). k/v
projections are computed data-parallel in a first launch (2 batches per
core) and redistributed through the host.

Perf-critical choices vs the v1 kernel:
  - fp16 everywhere instead of bf16 (same bytes, ~10x less rounding noise),
    except e / eT which stay bf16 (exp output can exceed fp16 range).
  - k_bias: t in [0,512) stored fp16, t in [512,1024) stored fp8-e3m4;
    v_bias: t-tiles 0-5 e3m4, 6-7 fp16. Mixed-dtype matmuls (fp16 lhsT x
    fp8 rhs) run at full stream rate, so this halves most of the dominant
    HBM traffic at a measured ~1.4e-2 total rel-err.
  - P1 (attn_2) and P4 (values_2) use PE column tiling: 4 query positions
    run concurrently in separate 32-column groups of the PE array
    (tile_position=(0,32g)), lifting the M=16 matmuls from 12.5% to ~50%
    array utilization and making the PSUM evacuations full-width.
  - values_2 is written out raw (with rowsums) and normalized + added on
    the host, killing the v2 row-shuffle pass entirely.
"""

import math
import numpy as np
import ml_dtypes

import concourse.bass as bass
import concourse.mybir as mybir
import concourse.tile as tile
from concourse import bacc
from concourse.masks import make_identity
from concourse.bass_utils import run_bass_kernel_spmd

B, S, H, D = 16, 1024, 1024, 128
NCORES = 8
SSL = S // NCORES          # query positions per core (128)
BS = B * S                 # 16384
HO = H // 128              # 8 h-chunks
TC = S // 128              # 8 t-chunks
PCHUNK = 512               # projection (b,t) chunk
NG = SSL // 4              # 32 col-tiled 4-s groups
KB16 = 512                 # k_bias cols stored fp16 (rest e3m4)
VB8 = 6                    # v_bias t-tiles stored e3m4 (rest fp16)

F16 = mybir.dt.float16
E3 = mybir.dt.float8e3
BF16 = mybir.dt.bfloat16
F32 = mybir.dt.float32

_cache = {}


def _build_proj_nc(reps=1):
    """Launch 1: data-parallel q/k/v projection; core handles 2 batches.
    Outputs qT/kT in [d, (b_local, t)] layout and v in [tp, b_local, tc, d].
    reps>1 repeats the whole body in-kernel (timing only)."""
    nc = bacc.Bacc()
    NB = 2
    NCH = NB * S // PCHUNK  # 4 chunks per tensor

    xTs = {k: nc.dram_tensor(f"{k}T", [H, NB * S], F16, kind="ExternalInput")
           for k in ("q", "k", "v")}
    Ws = {k: nc.dram_tensor(f"W{k}T", [H, D], F16, kind="ExternalInput")
          for k in ("q", "k", "v")}
    bs = {k: nc.dram_tensor(f"b{k}", [D], F32, kind="ExternalInput")
          for k in ("q", "k", "v")}
    qTo = nc.dram_tensor("qTo", [128, NB * S], F16, kind="ExternalOutput")
    kTo = nc.dram_tensor("kTo", [128, NB * S], F16, kind="ExternalOutput")
    vo = nc.dram_tensor("vo", [128, NB, TC, D], F16, kind="ExternalOutput")

    with tile.TileContext(nc) as tc:
        with (
            tc.tile_pool(name="const", bufs=1) as constp,
            tc.tile_pool(name="stream", bufs=3) as streamp,
            tc.tile_pool(name="evac", bufs=3) as evacp,
            tc.tile_pool(name="mmps", bufs=3, space="PSUM") as mmps,
            tc.tile_pool(name="tps", bufs=2, space="PSUM") as tps,
        ):
            ident = constp.tile([128, 128], F16)
            make_identity(nc, ident[:])
            w_sb, b_sb = {}, {}
            for k in ("q", "k", "v"):
                w_sb[k] = constp.tile([128, HO, D], F16, name=f"w_{k}", tag=f"w_{k}")
                nc.sync.dma_start(w_sb[k][:], Ws[k].rearrange("(ho p) d -> p ho d", p=128))
                b_sb[k] = constp.tile([128, 1], F32, name=f"b_{k}", tag=f"b_{k}")
                nc.sync.dma_start(b_sb[k][:], bs[k].rearrange("(o p) -> p o", p=128))
            vo_sb = constp.tile([128, NB, TC, D], F16)

            for rep in range(reps):
              for k in ("q", "k", "v"):
                src = xTs[k].rearrange("(ho p) n -> p ho n", p=128)
                for c in range(NCH):
                    xt = streamp.tile([128, HO, PCHUNK], F16, tag="xchunk")
                    nc.sync.dma_start(xt[:], src[:, :, c * PCHUNK:(c + 1) * PCHUNK])
                    ps = mmps.tile([128, PCHUNK], F32, tag="mm")
                    for ho in range(HO):
                        nc.tensor.matmul(ps[:], lhsT=w_sb[k][:, ho, :],
                                         rhs=xt[:, ho, :],
                                         start=(ho == 0), stop=(ho == HO - 1))
                    if k in ("q", "k"):
                        ev = evacp.tile([128, PCHUNK], F16, tag="ev")
                        nc.scalar.activation(ev[:], ps[:],
                                             mybir.ActivationFunctionType.Identity,
                                             bias=b_sb[k][:], scale=1.0)
                        dst = qTo if k == "q" else kTo
                        nc.sync.dma_start(dst[:, c * PCHUNK:(c + 1) * PCHUNK], ev[:])
                    else:
                        vt = evacp.tile([128, PCHUNK], F16, tag="vt")
                        nc.scalar.activation(vt[:], ps[:],
                                             mybir.ActivationFunctionType.Identity,
                                             bias=b_sb[k][:], scale=1.0)
                        bl = c // 2
                        for i in range(PCHUNK // 128):
                            tcg = (c % 2) * 4 + i
                            tp_ps = tps.tile([128, 128], F16, tag="tp")
                            nc.tensor.transpose(tp_ps[:], vt[:, i * 128:(i + 1) * 128],
                                                ident[:])
                            nc.vector.tensor_copy(out=vo_sb[:, bl, tcg, :], in_=tp_ps[:])
            nc.sync.dma_start(vo[:], vo_sb[:])
    nc.finalize()
    return nc


def _build_nc(mask_allones=True, reps=1):
    nc = bacc.Bacc()

    # ---- per-core inputs, all pre-projected/permuted host-side ----
    qT_in = nc.dram_tensor("qT_in", [128, B, SSL], F16, kind="ExternalInput")
    kT_in = nc.dram_tensor("kT_in", [128, B * S], F16, kind="ExternalInput")
    v_in = nc.dram_tensor("v_in", [128, B, TC, D], F16, kind="ExternalInput")
    # k_bias slice, pre-transposed to [d, s, t]; t split by precision
    kb16 = nc.dram_tensor("kb16", [128, SSL, KB16], F16, kind="ExternalInput")
    kb8 = nc.dram_tensor("kb8", [128, SSL, S - KB16], E3, kind="ExternalInput")
    # v_bias slice, pre-permuted to [tp, s, tc, d]; tc tiles split by precision
    vb8 = nc.dram_tensor("vb8", [128, SSL, VB8, D], E3, kind="ExternalInput")
    vb16 = nc.dram_tensor("vb16", [128, SSL, TC - VB8, D], F16, kind="ExternalInput")
    maskadd = nc.dram_tensor("maskadd", [B, S], F32, kind="ExternalInput")
    out_h = nc.dram_tensor("out", [B, SSL, D], F32, kind="ExternalOutput")
    v2o = nc.dram_tensor("v2o", [NG, 128, D], F32, kind="ExternalOutput")
    rso = nc.dram_tensor("rso", [SSL, B], F32, kind="ExternalOutput")

    with tile.TileContext(nc) as tc:
        with (
            tc.tile_pool(name="const", bufs=1) as constp,
            tc.tile_pool(name="big", bufs=1) as bigp,
            tc.tile_pool(name="stream", bufs=2) as streamp,
            tc.tile_pool(name="evac", bufs=3) as evacp,
        ):
            # ---- resident SBUF tensors ----
            kT_sb = bigp.tile([128, B, S], F16)           # [d, b, t]
            v_sb = bigp.tile([128, B, TC, 128], F16)      # [tp, b, tc, d]
            qT_sb = bigp.tile([128, B, SSL], F16)         # [d, b, s]
            a2buf = bigp.tile([128, B, S], F16)           # [s, b, t]
            eT_sb = bigp.tile([128, TC, B, SSL], BF16)    # [tp, tc, b, s]
            outbuf = bigp.tile([128, B, D], F32)          # [s, b, d]
            rowsum = bigp.tile([128, B], F32)
            recip = bigp.tile([128, B], F32)

            ident = constp.tile([128, 128], BF16)
            make_identity(nc, ident[:])
            ident16 = constp.tile([128, 128], F16)
            make_identity(nc, ident16[:])
            if not mask_allones:
                mask4 = constp.tile([128, S], F32)
                for g in range(4):
                    nc.scalar.dma_start(mask4[32 * g:32 * g + 16, :], maskadd[:, :])

            for rep in range(reps):
              # qT first (needed by P1); big kT/v preloads on the ACT HWDGE
              # queue so the kb stream isn't queued behind them
              nc.sync.dma_start(qT_sb[:], qT_in[:, :, :])
              nc.scalar.dma_start(kT_sb.rearrange("p b t -> p (b t)")[:], kT_in[:, :])
              nc.scalar.dma_start(v_sb[:], v_in[:])

              # ========== P1: attn_2, col-tiled 4 s at a time ==========
              # a2[b,t] = sum_d q[b,s,d]*kb[s,t,d]; group G handles s=4G..4G+3,
              # each s in its own 32-col group of the PE array.
              shuffle_engines = [nc.gpsimd, nc.scalar, nc.sync]
              with tc.tile_pool(name=f"a2ps{rep}", bufs=3, space="PSUM") as a2ps:
                for G in range(NG):
                    s0 = 4 * G
                    kt16 = streamp.tile([128, 4, KB16], F16, tag="kbt16", bufs=2)
                    nc.sync.dma_start(kt16[:], kb16[:, s0:s0 + 4, :])
                    kt8 = streamp.tile([128, 4, S - KB16], E3, tag="kbt8", bufs=2)
                    nc.sync.dma_start(kt8[:], kb8[:, s0:s0 + 4, :])
                    ps = a2ps.tile([128, S], F32, tag="a2")
                    for g in range(4):
                        s = s0 + g
                        nc.tensor.matmul(ps[32 * g:32 * g + 16, 0:KB16],
                                         lhsT=qT_sb[:, :, s], rhs=kt16[:, g, :],
                                         start=True, stop=True,
                                         tile_position=(0, 32 * g))
                        nc.tensor.matmul(ps[32 * g:32 * g + 16, KB16:S],
                                         lhsT=qT_sb[:, :, s], rhs=kt8[:, g, :],
                                         start=True, stop=True,
                                         tile_position=(0, 32 * g))
                    ev = evacp.tile([128, S], F16, tag="a2evac")
                    if mask_allones:
                        if G % 2 == 0:
                            nc.vector.tensor_copy(out=ev[:], in_=ps[:])
                        else:
                            nc.scalar.copy(ev[:], ps[:])
                    else:
                        nc.vector.tensor_add(out=ev[:], in0=ps[:], in1=mask4[:])
                    # row-shuffle each s into partition s of a2buf
                    for g in range(4):
                        eng = shuffle_engines[(4 * G + g) % 3]
                        eng.dma_start(a2buf[s0 + g:s0 + g + 1, :, :],
                                      ev[32 * g:32 * g + 16, :])

              # ========== P3a: scores + softmax + eT, per b ==========
              with (
                tc.tile_pool(name=f"scps{rep}", bufs=2, space="PSUM") as scps,
                tc.tile_pool(name=f"tps2{rep}", bufs=2, space="PSUM") as tps2,
              ):
                for b in range(B):
                    ps = scps.tile([128, S], F32, tag="sc")
                    for h in range(2):
                        sl = slice(h * 512, (h + 1) * 512)
                        nc.tensor.matmul(ps[:, sl], lhsT=qT_sb[:, b, :],
                                         rhs=kT_sb[:, b, sl], start=True, stop=False)
                        nc.tensor.matmul(ps[:, sl], lhsT=ident16[:],
                                         rhs=a2buf[:, b, sl], start=False, stop=True)
                    e_sb = evacp.tile([128, S], BF16, tag="e", bufs=2)
                    nc.scalar.activation(e_sb[:], ps[:],
                                         mybir.ActivationFunctionType.Exp,
                                         bias=0.0, scale=1.0,
                                         accum_out=rowsum[:, b:b + 1])
                    for t in range(TC):
                        tp_ps = tps2.tile([128, 128], BF16, tag="tp2")
                        nc.tensor.transpose(tp_ps[:], e_sb[:, t * 128:(t + 1) * 128],
                                            ident[:])
                        nc.vector.tensor_copy(out=eT_sb[:, t, b, :], in_=tp_ps[:])
                nc.vector.reciprocal(recip[:], rowsum[:])
                nc.gpsimd.dma_start(rso[:, :], rowsum[:])

              # ========== P4: values_2 raw, col-tiled 4 s at a time ==========
              # v2[b,d] = sum_t e[b,s,t]*vb[s,t,d]; normalized + added on host.
              with tc.tile_pool(name=f"v2ps{rep}", bufs=4, space="PSUM") as v2ps:
                for G in range(NG):
                    s0 = 4 * G
                    vt8 = streamp.tile([128, 4, VB8, D], E3, tag="vbt8", bufs=2)
                    nc.sync.dma_start(vt8[:], vb8[:, s0:s0 + 4, :, :])
                    vt16 = streamp.tile([128, 4, TC - VB8, D], F16, tag="vbt16", bufs=2)
                    nc.sync.dma_start(vt16[:], vb16[:, s0:s0 + 4, :, :])
                    ps = v2ps.tile([128, D], F32, tag="v2")
                    for g in range(4):
                        s = s0 + g
                        for t in range(TC):
                            rhs = (vt8[:, g, t, :] if t < VB8
                                   else vt16[:, g, t - VB8, :])
                            nc.tensor.matmul(ps[32 * g:32 * g + 16, :],
                                             lhsT=eT_sb[:, t, :, s], rhs=rhs,
                                             start=(t == 0), stop=(t == TC - 1),
                                             tile_position=(0, 32 * g))
                    ev = evacp.tile([128, D], F32, tag="v2evac")
                    if G % 2 == 0:
                        nc.vector.tensor_copy(out=ev[:], in_=ps[:])
                    else:
                        nc.scalar.copy(ev[:], ps[:])
                    eng = shuffle_engines[G % 2]
                    eng.dma_start(v2o[G], ev[:])

              # ========== P3b: values_1 + normalize + out ==========
              with tc.tile_pool(name=f"ops{rep}", bufs=2, space="PSUM") as ops:
                for b in range(B):
                    ps = ops.tile([128, D], F32, tag="o")
                    for t in range(TC):
                        nc.tensor.matmul(ps[:], lhsT=eT_sb[:, t, b, :],
                                         rhs=v_sb[:, b, t, :],
                                         start=(t == 0), stop=(t == TC - 1))
                    nc.scalar.activation(outbuf[:, b, :], ps[:],
                                         mybir.ActivationFunctionType.Copy,
                                         bias=0.0, scale=recip[:, b:b + 1])
                    nc.sync.dma_start(out_h[b].rearrange("s d -> s d"),
                                      outbuf[:, b, :])

    nc.finalize()
    return nc


def _prep_proj_inputs(query, key, value, Wq, bq, Wk, bk, Wv, bv):
    scale = 1.0 / math.sqrt(D)
    f16 = np.float16
    WqTs = np.ascontiguousarray((Wq.T * scale)).astype(f16)
    WkT = np.ascontiguousarray(Wk.T).astype(f16)
    WvT = np.ascontiguousarray(Wv.T).astype(f16)
    bqs = (bq * scale).astype(np.float32)
    in_maps = []
    for c in range(NCORES):
        bsl = slice(2 * c, 2 * c + 2)
        m = dict(WqT=WqTs, WkT=WkT, WvT=WvT,
                 bq=bqs, bk=bk.astype(np.float32), bv=bv.astype(np.float32))
        for nm, x in (("qT", query), ("kT", key), ("vT", value)):
            m[nm] = np.ascontiguousarray(
                x[bsl].transpose(2, 0, 1).reshape(H, 2 * S)).astype(f16)
        in_maps.append(m)
    return in_maps


def _prep_attn_inputs(proj_results, mask, k_bias, v_bias):
    f16 = np.float16
    e3 = ml_dtypes.float8_e3m4
    qT_full = np.concatenate(  # [128, B, S]
        [r["qTo"].reshape(128, 2, S) for r in proj_results], axis=1)
    kT_full = np.concatenate(
        [r["kTo"].reshape(128, 2, S) for r in proj_results], axis=1)
    v_full = np.concatenate(  # [128, B, TC, D]
        [r["vo"] for r in proj_results], axis=1)
    kT_in = np.ascontiguousarray(kT_full.reshape(128, B * S))
    v_in = np.ascontiguousarray(v_full)
    maskadd = np.where(mask == 0, np.float32(-30000.0),
                       np.float32(0.0)).astype(np.float32)

    in_maps = []
    for c in range(NCORES):
        ssl = slice(c * SSL, (c + 1) * SSL)
        qT_in = np.ascontiguousarray(qT_full[:, :, ssl])
        kbT = k_bias[ssl].transpose(2, 0, 1)          # [d, s, t]
        kb16c = np.ascontiguousarray(kbT[:, :, :KB16]).astype(f16)
        kb8c = np.ascontiguousarray(kbT[:, :, KB16:]).astype(e3)
        vbp = v_bias[ssl].reshape(SSL, TC, 128, D).transpose(2, 0, 1, 3)
        vb8c = np.ascontiguousarray(vbp[:, :, :VB8, :]).astype(e3)
        vb16c = np.ascontiguousarray(vbp[:, :, VB8:, :]).astype(f16)
        in_maps.append(dict(qT_in=qT_in, kT_in=kT_in, v_in=v_in,
                            kb16=kb16c, kb8=kb8c, vb8=vb8c, vb16=vb16c,
                            maskadd=maskadd))
    return in_maps


def kernel(**inputs):
    ins = {k: np.asarray(v) for k, v in inputs.items()}
    allones = bool((ins["mask"] != 0).all())
    if "nc_proj" not in _cache:
        _cache["nc_proj"] = _build_proj_nc()
    key = f"nc{int(allones)}"
    if key not in _cache:
        _cache[key] = _build_nc(mask_allones=allones)
    nc = _cache[key]
    _cache["nc"] = nc

    proj_maps = _prep_proj_inputs(
        ins["query"], ins["key"], ins["value"], ins["Wq"], ins["bq"],
        ins["Wk"], ins["bk"], ins["Wv"], ins["bv"])
    _cache["proj_in_maps"] = proj_maps
    res1 = run_bass_kernel_spmd(_cache["nc_proj"], proj_maps,
                                core_ids=list(range(NCORES)))
    in_maps = _prep_attn_inputs(res1.results, ins["mask"], ins["k_bias"],
                                ins["v_bias"])
    _cache["attn_in_maps"] = in_maps
    res = run_bass_kernel_spmd(nc, in_maps, core_ids=list(range(NCORES)))

    # assemble: out = normalized values_1; add host-normalized values_2
    out = np.concatenate([r["out"] for r in res.results], axis=1)  # [B,S,D]
    for c in range(NCORES):
        v2 = res.results[c]["v2o"].reshape(NG, 4, 32, D)[:, :, :B, :]
        v2 = v2.transpose(2, 0, 1, 3).reshape(B, SSL, D)       # [b, s_local, d]
        rs = res.results[c]["rso"]                              # [s_local, b]
        out[:, c * SSL:(c + 1) * SSL, :] += v2 / rs.T[:, :, None]
    return out
